# revision 5
# baseline (speedup 1.0000x reference)
"""2-layer GCN (EvidentialGNN) on 8 Trainium2 NeuronCores.

Math (per GCNConv with self-loops and symmetric normalization):
    deg[n]  = in-degree(n) + 1          (self loop)
    dinv    = deg ** -0.5
    out[d]  = dinv[d] * sum_{e:(s->d)} (dinv[s] * h[s])  + b
with the edge set including the self loop (d->d).

Key factorization: pre-scale rows h_s[n] = dinv[n] * h[n] BEFORE the
exchange; then out[d] = dinv[d] * sum_e h_s[src_e] + b.  Per-edge work
reduces to a pure gather + segmented-sum (no per-edge multiplies).

Distribution (graph/data parallel over 8 cores):
  - nodes sharded by contiguous range (6250/core, padded to 6272).
  - core c computes h1 = x@W1 for its rows, scales by dinv, AllGathers
    the scaled table; then aggregates its incoming edges (dst in shard)
    by gathering source rows with dma_gather and accumulating per
    128-dst tile in PSUM via one-hot matmuls (edges dst-sorted on host).
  - layer 2 repeats the pattern at width 64.

Host-side prep is pure integer index manipulation (shard bucketing,
dst-sorting, padding, degree counting); all float math runs on device.
"""

import numpy as np

import concourse.bacc as bacc
import concourse.bass as bass
import concourse.mybir as mybir
import concourse.tile as tile
from concourse.bass_utils import run_bass_kernel_spmd
from concourse.masks import make_identity

P = 128
N_NODES = 50000
N_EDGES = 800000
NCORES = 8
IN_DIM = 512
HID = 256
NCLS = 64
SH = N_NODES // NCORES  # 6250 rows per core
NT = (SH + P - 1) // P  # 49 dst tiles per core
SHP = NT * P  # 6272 padded rows per core
SPLIT_CORE = 4  # cores 0..3 feed the "lo" gather table
LO_ROWS = SPLIT_CORE * SHP  # 25088 (< 2**15, int16-addressable)
HI_ROWS = NCORES * SHP - LO_ROWS

F32 = mybir.dt.float32
I16 = mybir.dt.int16

# Results of the last kernel() call (for test harness introspection).
LAST_RESULTS = None


def _prep(edge_index):
    """Pure-index host prep.

    Returns:
      deg:   [N_NODES] float32 (in-degree + 1)
      chunks: list over dst-tile t of (ch_lo, ch_hi) 128-edge chunk counts
              (shared by all cores: max over cores, padded)
      idx_w: [NCORES, 16, PADTOT//16] int16 gather indices (wrapped)
      offs_w:[NCORES, 128, NCH] float32 local dst offsets (-1 = padding)
    """
    src = np.asarray(edge_index[0]).astype(np.int64)
    dst = np.asarray(edge_index[1]).astype(np.int64)
    loops = np.arange(N_NODES, dtype=np.int64)
    s_all = np.concatenate([src, loops])
    d_all = np.concatenate([dst, loops])

    deg = np.bincount(d_all, minlength=N_NODES).astype(np.float32)

    core_of = d_all // SH
    # per-core edge lists sorted by local dst
    per_core = []
    for c in range(NCORES):
        m = core_of == c
        sc = s_all[m]
        dc = d_all[m] - c * SH
        order = np.argsort(dc, kind="stable")
        per_core.append((sc[order], dc[order]))

    # group edges by (tile, half); count per core
    cnt = np.zeros((NCORES, NT, 2), np.int64)
    grouped = []  # [c][t][half] -> (table_idx int array, local_off int array)
    for c in range(NCORES):
        sc, dc = per_core[c]
        t_of = dc // P
        half_of = (sc >= SPLIT_CORE * SH).astype(np.int64)
        table_row = (sc // SH) * SHP + (sc % SH)
        table_row = table_row - half_of * LO_ROWS
        off = dc - t_of * P
        g = [[None, None] for _ in range(NT)]
        for t in range(NT):
            mt = t_of == t
            for h in (0, 1):
                mh = mt & (half_of == h)
                g[t][h] = (table_row[mh], off[mh])
                cnt[c, t, h] = mh.sum()
        grouped.append(g)

    cmax = cnt.max(axis=0)  # [NT, 2]
    chunks = [
        (int(-(-cmax[t, 0] // P)), int(-(-cmax[t, 1] // P))) for t in range(NT)
    ]
    nch = sum(a + b for a, b in chunks)
    padtot = nch * P

    idx_flat = np.zeros((NCORES, padtot), np.int16)
    offs_flat = np.full((NCORES, padtot), -1.0, np.float32)
    for c in range(NCORES):
        pos = 0
        for t in range(NT):
            for h in (0, 1):
                rows, off = grouped[c][t][h]
                L = chunks[t][h] * P
                n = len(rows)
                idx_flat[c, pos : pos + n] = rows.astype(np.int16)
                offs_flat[c, pos : pos + n] = off.astype(np.float32)
                pos += L
        assert pos == padtot

    idx_w = np.ascontiguousarray(
        idx_flat.reshape(NCORES, padtot // 16, 16).transpose(0, 2, 1)
    )
    offs_w = np.ascontiguousarray(
        offs_flat.reshape(NCORES, nch, P).transpose(0, 2, 1)
    )
    return deg, chunks, idx_w, offs_w


def _build(chunks):
    """Build the SPMD Bass program (shared across all 8 cores)."""
    nch = sum(a + b for a, b in chunks)
    padtot = nch * P
    max_cht = max(a + b for a, b in chunks)

    nc = bacc.Bacc("TRN2", target_bir_lowering=False, debug=False, num_devices=NCORES)

    x_in = nc.dram_tensor("x", [SHP, IN_DIM], F32, kind="ExternalInput")
    w1_in = nc.dram_tensor("w1", [IN_DIM, HID], F32, kind="ExternalInput")
    b1_in = nc.dram_tensor("b1", [1, HID], F32, kind="ExternalInput")
    w2_in = nc.dram_tensor("w2", [HID, NCLS], F32, kind="ExternalInput")
    b2_in = nc.dram_tensor("b2", [1, NCLS], F32, kind="ExternalInput")
    deg_in = nc.dram_tensor("deg", [P, NT], F32, kind="ExternalInput")
    idx_in = nc.dram_tensor("idx", [16, padtot // 16], I16, kind="ExternalInput")
    offs_in = nc.dram_tensor("offs", [P, nch], F32, kind="ExternalInput")
    iota_in = nc.dram_tensor("iota", [1, P], F32, kind="ExternalInput")

    h_out = nc.dram_tensor("h_out", [SHP, HID], F32, kind="ExternalOutput")
    ev_out = nc.dram_tensor("ev_out", [SHP, NCLS], F32, kind="ExternalOutput")

    ag1_in = nc.dram_tensor("ag1_in", [SHP, HID], F32)
    ag1_out = nc.dram_tensor("ag1_out", [NCORES * SHP, HID], F32)
    ag2_in = nc.dram_tensor("ag2_in", [SHP, NCLS], F32)
    ag2_out = nc.dram_tensor("ag2_out", [NCORES * SHP, NCLS], F32)

    rg = [list(range(NCORES))]

    with tile.TileContext(nc) as tc:
        with (
            tc.tile_pool(name="const", bufs=1) as cpool,
            tc.tile_pool(name="xw", bufs=3) as xwpool,
            tc.tile_pool(name="work", bufs=3) as wpool,
            tc.tile_pool(name="msgs", bufs=2) as mpool,
            tc.tile_pool(name="oh", bufs=6) as ohpool,
            tc.tile_pool(name="ptr", bufs=2, space="PSUM") as ptr,
            tc.tile_pool(name="pmm", bufs=2, space="PSUM") as pmm,
        ):
            # ---- constants
            ident = cpool.tile([P, P], F32)
            make_identity(nc, ident[:])
            iota_b = cpool.tile([P, P], F32)
            nc.gpsimd.dma_start(out=iota_b[:], in_=iota_in[:].to_broadcast([P, P]))
            b1_b = cpool.tile([P, HID], F32)
            nc.gpsimd.dma_start(out=b1_b[:], in_=b1_in[:].to_broadcast([P, HID]))
            b2_b = cpool.tile([P, NCLS], F32)
            nc.gpsimd.dma_start(out=b2_b[:], in_=b2_in[:].to_broadcast([P, NCLS]))
            w1_sb = cpool.tile([P, IN_DIM // P, HID], F32)
            for k in range(IN_DIM // P):
                nc.sync.dma_start(
                    out=w1_sb[:, k, :], in_=w1_in[k * P : (k + 1) * P, :]
                )
            w2_sb = cpool.tile([P, HID // P, NCLS], F32)
            for k in range(HID // P):
                nc.sync.dma_start(
                    out=w2_sb[:, k, :], in_=w2_in[k * P : (k + 1) * P, :]
                )
            deg_sb = cpool.tile([P, NT], F32)
            nc.sync.dma_start(out=deg_sb[:], in_=deg_in[:])
            dinv_r = cpool.tile([P, NT], F32)
            nc.vector.reciprocal(out=dinv_r[:], in_=deg_sb[:])
            dinv = cpool.tile([P, NT], F32)
            nc.scalar.activation(
                out=dinv[:], in_=dinv_r[:], func=mybir.ActivationFunctionType.Sqrt
            )
            idx_sb = cpool.tile([P, padtot // 16], I16)
            for rep in range(8):
                nc.sync.dma_start(
                    out=idx_sb[rep * 16 : (rep + 1) * 16, :], in_=idx_in[:]
                )
            offs_sb = cpool.tile([P, nch], F32)
            nc.sync.dma_start(out=offs_sb[:], in_=offs_in[:])

            # ---- phase 1: h1s = dinv * (x @ W1), write to ag1_in
            for t in range(NT):
                x_t = xwpool.tile([P, IN_DIM], F32, tag="x")
                nc.sync.dma_start(out=x_t[:], in_=x_in[t * P : (t + 1) * P, :])
                xT = xwpool.tile([P, IN_DIM // P, P], F32, tag="xT")
                for k in range(IN_DIM // P):
                    tp = ptr.tile([P, P], F32, tag="tr")
                    nc.tensor.transpose(
                        out=tp[:], in_=x_t[:, k * P : (k + 1) * P], identity=ident[:]
                    )
                    nc.scalar.activation(
                        out=xT[:, k, :],
                        in_=tp[:],
                        func=mybir.ActivationFunctionType.Copy,
                    )
                hp = pmm.tile([P, HID], F32, tag="mm")
                for k in range(IN_DIM // P):
                    nc.tensor.matmul(
                        out=hp[:],
                        lhsT=xT[:, k, :],
                        rhs=w1_sb[:, k, :],
                        start=(k == 0),
                        stop=(k == IN_DIM // P - 1),
                    )
                h1s = xwpool.tile([P, HID], F32, tag="h1s")
                nc.scalar.activation(
                    out=h1s[:],
                    in_=hp[:],
                    func=mybir.ActivationFunctionType.Copy,
                    scale=dinv[:, t : t + 1],
                )
                nc.sync.dma_start(
                    out=ag1_in[t * P : (t + 1) * P, :], in_=h1s[:]
                )

            nc.gpsimd.collective_compute(
                "AllGather",
                mybir.AluOpType.bypass,
                replica_groups=rg,
                ins=[ag1_in.ap().opt()],
                outs=[ag1_out.ap().opt()],
            )

            # ---- phase 2: L1 aggregation + epilogue + L2 transform
            seg_col = 0  # running idx column (each chunk = 8 cols of idx_sb)
            seg_ch = 0  # running chunk index (offs column)
            lo_tbl = ag1_out[0:LO_ROWS, :]
            hi_tbl = ag1_out[LO_ROWS : LO_ROWS + HI_ROWS, :]
            seg_pos = []  # remember (col, ch) per tile for phase 3
            for t in range(NT):
                ch_lo, ch_hi = chunks[t]
                cht = ch_lo + ch_hi
                seg_pos.append((seg_col, seg_ch))
                msgs = mpool.tile([P, max_cht, HID], F32, tag="m1")
                if ch_lo:
                    nc.gpsimd.dma_gather(
                        out_ap=msgs[:, 0:ch_lo, :],
                        in_ap=lo_tbl,
                        idxs_ap=idx_sb[:, seg_col : seg_col + ch_lo * 8],
                        num_idxs=ch_lo * P,
                        num_idxs_reg=ch_lo * P,
                        elem_size=HID,
                        single_packet=False,
                    )
                if ch_hi:
                    nc.gpsimd.dma_gather(
                        out_ap=msgs[:, ch_lo:cht, :],
                        in_ap=hi_tbl,
                        idxs_ap=idx_sb[
                            :, seg_col + ch_lo * 8 : seg_col + cht * 8
                        ],
                        num_idxs=ch_hi * P,
                        num_idxs_reg=ch_hi * P,
                        elem_size=HID,
                        single_packet=False,
                    )
                agg = pmm.tile([P, HID], F32, tag="mm")
                for j in range(cht):
                    oh = ohpool.tile([P, P], F32, tag="oh")
                    nc.vector.tensor_tensor(
                        out=oh[:],
                        in0=offs_sb[:, seg_ch + j : seg_ch + j + 1].to_broadcast(
                            [P, P]
                        ),
                        in1=iota_b[:],
                        op=mybir.AluOpType.is_equal,
                    )
                    nc.tensor.matmul(
                        out=agg[:],
                        lhsT=oh[:],
                        rhs=msgs[:, j, :],
                        start=(j == 0),
                        stop=(j == cht - 1),
                    )
                pre = wpool.tile([P, HID], F32, tag="pre")
                nc.vector.scalar_tensor_tensor(
                    out=pre[:],
                    in0=agg[:],
                    scalar=dinv[:, t : t + 1],
                    in1=b1_b[:],
                    op0=mybir.AluOpType.mult,
                    op1=mybir.AluOpType.add,
                )
                h_t = wpool.tile([P, HID], F32, tag="ht")
                nc.scalar.activation(
                    out=h_t[:], in_=pre[:], func=mybir.ActivationFunctionType.Relu
                )
                nc.sync.dma_start(out=h_out[t * P : (t + 1) * P, :], in_=h_t[:])
                # L2 transform: h2s = dinv * (h @ W2)
                hT = wpool.tile([P, HID // P, P], F32, tag="hT")
                for k in range(HID // P):
                    tp = ptr.tile([P, P], F32, tag="tr")
                    nc.tensor.transpose(
                        out=tp[:], in_=h_t[:, k * P : (k + 1) * P], identity=ident[:]
                    )
                    nc.scalar.activation(
                        out=hT[:, k, :],
                        in_=tp[:],
                        func=mybir.ActivationFunctionType.Copy,
                    )
                h2p = pmm.tile([P, NCLS], F32, tag="mm2")
                for k in range(HID // P):
                    nc.tensor.matmul(
                        out=h2p[:],
                        lhsT=hT[:, k, :],
                        rhs=w2_sb[:, k, :],
                        start=(k == 0),
                        stop=(k == HID // P - 1),
                    )
                h2s = wpool.tile([P, NCLS], F32, tag="h2s")
                nc.scalar.activation(
                    out=h2s[:],
                    in_=h2p[:],
                    func=mybir.ActivationFunctionType.Copy,
                    scale=dinv[:, t : t + 1],
                )
                nc.sync.dma_start(
                    out=ag2_in[t * P : (t + 1) * P, :], in_=h2s[:]
                )
                seg_col += cht * 8
                seg_ch += cht

            nc.gpsimd.collective_compute(
                "AllGather",
                mybir.AluOpType.bypass,
                replica_groups=rg,
                ins=[ag2_in.ap().opt()],
                outs=[ag2_out.ap().opt()],
            )

            # ---- phase 3: L2 aggregation + softplus
            lo2 = ag2_out[0:LO_ROWS, :]
            hi2 = ag2_out[LO_ROWS : LO_ROWS + HI_ROWS, :]
            for t in range(NT):
                ch_lo, ch_hi = chunks[t]
                cht = ch_lo + ch_hi
                seg_col, seg_ch = seg_pos[t]
                msgs = mpool.tile([P, max_cht, NCLS], F32, tag="m2")
                if ch_lo:
                    nc.gpsimd.dma_gather(
                        out_ap=msgs[:, 0:ch_lo, :],
                        in_ap=lo2,
                        idxs_ap=idx_sb[:, seg_col : seg_col + ch_lo * 8],
                        num_idxs=ch_lo * P,
                        num_idxs_reg=ch_lo * P,
                        elem_size=NCLS,
                        single_packet=False,
                    )
                if ch_hi:
                    nc.gpsimd.dma_gather(
                        out_ap=msgs[:, ch_lo:cht, :],
                        in_ap=hi2,
                        idxs_ap=idx_sb[
                            :, seg_col + ch_lo * 8 : seg_col + cht * 8
                        ],
                        num_idxs=ch_hi * P,
                        num_idxs_reg=ch_hi * P,
                        elem_size=NCLS,
                        single_packet=False,
                    )
                agg2 = pmm.tile([P, NCLS], F32, tag="mm2")
                for j in range(cht):
                    oh = ohpool.tile([P, P], F32, tag="oh")
                    nc.vector.tensor_tensor(
                        out=oh[:],
                        in0=offs_sb[:, seg_ch + j : seg_ch + j + 1].to_broadcast(
                            [P, P]
                        ),
                        in1=iota_b[:],
                        op=mybir.AluOpType.is_equal,
                    )
                    nc.tensor.matmul(
                        out=agg2[:],
                        lhsT=oh[:],
                        rhs=msgs[:, j, :],
                        start=(j == 0),
                        stop=(j == cht - 1),
                    )
                pre2 = wpool.tile([P, NCLS], F32, tag="pre2")
                nc.vector.scalar_tensor_tensor(
                    out=pre2[:],
                    in0=agg2[:],
                    scalar=dinv[:, t : t + 1],
                    in1=b2_b[:],
                    op0=mybir.AluOpType.mult,
                    op1=mybir.AluOpType.add,
                )
                # softplus(x) = ln(exp(x) + 1); Exp and Ln share one ACT table
                evx = wpool.tile([P, NCLS], F32, tag="evx")
                nc.scalar.activation(
                    out=evx[:],
                    in_=pre2[:],
                    func=mybir.ActivationFunctionType.Exp,
                )
                ev = wpool.tile([P, NCLS], F32, tag="ev")
                nc.scalar.activation(
                    out=ev[:],
                    in_=evx[:],
                    func=mybir.ActivationFunctionType.Ln,
                    bias=1.0,
                )
                nc.sync.dma_start(out=ev_out[t * P : (t + 1) * P, :], in_=ev[:])

    nc.compile()
    return nc


def kernel(x, edge_index, W1, b1, W2, b2, trace=False):
    global LAST_RESULTS
    x = np.asarray(x, dtype=np.float32)
    W1 = np.asarray(W1, dtype=np.float32)
    b1 = np.asarray(b1, dtype=np.float32)
    W2 = np.asarray(W2, dtype=np.float32)
    b2 = np.asarray(b2, dtype=np.float32)

    deg, chunks, idx_w, offs_w = _prep(edge_index)
    nc = _build(chunks)

    iota = np.arange(P, dtype=np.float32).reshape(1, P)
    in_maps = []
    for c in range(NCORES):
        x_pad = np.zeros((SHP, IN_DIM), np.float32)
        x_pad[:SH] = x[c * SH : (c + 1) * SH]
        deg_pad = np.ones(SHP, np.float32)
        deg_pad[:SH] = deg[c * SH : (c + 1) * SH]
        deg_w = np.ascontiguousarray(deg_pad.reshape(NT, P).T)
        in_maps.append(
            dict(
                x=x_pad,
                w1=W1,
                b1=b1.reshape(1, HID),
                w2=W2,
                b2=b2.reshape(1, NCLS),
                deg=deg_w,
                idx=idx_w[c],
                offs=offs_w[c],
                iota=iota,
            )
        )

    res = run_bass_kernel_spmd(
        nc,
        in_maps,
        core_ids=list(range(NCORES)),
        trace=trace,
        tmpdir="/tmp/prof_kernel" if trace else None,
    )
    LAST_RESULTS = res

    h = np.concatenate([res.results[c]["h_out"][:SH] for c in range(NCORES)], axis=0)
    ev = np.concatenate(
        [res.results[c]["ev_out"][:SH] for c in range(NCORES)], axis=0
    )
    return ev, h


# revision 9
# speedup vs baseline: 1.9733x; 1.9733x over previous
"""2-layer GCN (EvidentialGNN) on 8 Trainium2 NeuronCores.

Math (per GCNConv with self-loops and symmetric normalization):
    deg[n]  = in-degree(n) + 1          (self loop)
    dinv    = deg ** -0.5
    out[d]  = dinv[d] * ( sum_{e:(s->d)} (dinv[s] * h[s]) + dinv[d]*h[d] ) + b

Key factorization: pre-scale rows h_s[n] = dinv[n] * h[n] BEFORE the
exchange; then out[d] = dinv[d] * (sum_e h_s[src_e] + h_s[d]) + b.
Per-edge work reduces to a pure gather + segmented sum; the self-loop
term uses the locally available h_s tile (no gather).

Distribution (graph/data parallel over 8 cores):
  - nodes sharded by contiguous range (6250/core, padded to 6272).
  - core c computes h1 = x@W1 for its rows (weights replicated, x
    pre-transposed on host so no PE transposes), scales by dinv, casts
    to bf16 and AllGathers the scaled message table; then aggregates
    its incoming edges (dst in shard) by gathering source rows with
    dma_gather (SWDGE queues 1-3 round-robin for parallel descriptor
    generation) and accumulating per 128-dst tile in fp32 PSUM via
    one-hot bf16 matmuls (edges dst-sorted on host).
  - layer 2 repeats at width 64 (table padded to 128 bf16 cols to meet
    the gather's 256B row-granularity).

Host-side prep is pure integer index manipulation (shard bucketing,
dst-sorting, padding, degree counting) plus layout transposes; all
float math runs on device.
"""

import numpy as np
import ml_dtypes

import concourse.bacc as bacc
import concourse.bass as bass
import concourse.mybir as mybir
import concourse.tile as tile
from concourse.bass_utils import run_bass_kernel_spmd

P = 128
N_NODES = 50000
NCORES = 8
IN_DIM = 512
HID = 256
NCLS = 64
NC2 = 2 * NCLS  # L2 table padded to 128 cols (gather needs 256B rows)
SH = N_NODES // NCORES  # 6250 rows per core
NT = (SH + P - 1) // P  # 49 dst tiles per core
SHP = NT * P  # 6272 padded rows per core
SPLIT_CORE = 4  # cores 0..3 feed the "lo" gather table
LO_ROWS = SPLIT_CORE * SHP  # 25088 (< 2**15, int16-addressable)
HI_ROWS = NCORES * SHP - LO_ROWS

F32 = mybir.dt.float32
BF16 = mybir.dt.bfloat16
I16 = mybir.dt.int16
AF = mybir.ActivationFunctionType

# Results of the last kernel() call (for test harness introspection).
LAST_RESULTS = None


def _prep(edge_index):
    """Pure-index host prep (no self-loop edges; handled in epilogue).

    Returns:
      deg:    [N_NODES] float32 (in-degree + 1)
      chunks: list over dst-tile t of (ch_lo, ch_hi) 128-edge chunk counts
              (shared by all cores: max over cores)
      idx_w:  [NCORES, 16, PADTOT//16] int16 gather indices (wrapped);
              padding = 0 (gathers row 0, zeroed by the one-hot)
      offs_w: [NCORES, 128, NCH] bfloat16 local dst offsets (-1 = padding)
    """
    src = np.asarray(edge_index[0]).astype(np.int64)
    dst = np.asarray(edge_index[1]).astype(np.int64)

    deg = (np.bincount(dst, minlength=N_NODES) + 1).astype(np.float32)

    core_of = dst // SH
    per_core = []
    for c in range(NCORES):
        m = core_of == c
        sc = src[m]
        dc = dst[m] - c * SH
        order = np.argsort(dc, kind="stable")
        per_core.append((sc[order], dc[order]))

    cnt = np.zeros((NCORES, NT, 2), np.int64)
    grouped = []
    for c in range(NCORES):
        sc, dc = per_core[c]
        t_of = dc // P
        half_of = (sc >= SPLIT_CORE * SH).astype(np.int64)
        table_row = (sc // SH) * SHP + (sc % SH) - half_of * LO_ROWS
        off = dc - t_of * P
        g = [[None, None] for _ in range(NT)]
        for t in range(NT):
            mt = t_of == t
            for h in (0, 1):
                mh = mt & (half_of == h)
                g[t][h] = (table_row[mh], off[mh])
                cnt[c, t, h] = mh.sum()
        grouped.append(g)

    cmax = cnt.max(axis=0)  # [NT, 2]
    chunks = [
        (int(-(-cmax[t, 0] // P)), int(-(-cmax[t, 1] // P))) for t in range(NT)
    ]
    nch = sum(a + b for a, b in chunks)
    padtot = nch * P

    idx_flat = np.zeros((NCORES, padtot), np.int16)
    offs_flat = np.full((NCORES, padtot), -1.0, np.float32)
    for c in range(NCORES):
        pos = 0
        for t in range(NT):
            for h in (0, 1):
                rows, off = grouped[c][t][h]
                L = chunks[t][h] * P
                n = len(rows)
                # padding keeps idx 0 (valid row, zeroed by one-hot off=-1)
                idx_flat[c, pos : pos + n] = rows.astype(np.int16)
                offs_flat[c, pos : pos + n] = off.astype(np.float32)
                pos += L
        assert pos == padtot

    idx_w = np.ascontiguousarray(
        idx_flat.reshape(NCORES, padtot // 16, 16).transpose(0, 2, 1)
    )
    offs_w = np.ascontiguousarray(
        offs_flat.reshape(NCORES, nch, P).transpose(0, 2, 1)
    ).astype(ml_dtypes.bfloat16)
    return deg, chunks, idx_w, offs_w


def _build(chunks):
    """Build the SPMD Bass program (shared across all 8 cores)."""
    nch = sum(a + b for a, b in chunks)
    padtot = nch * P
    max_cht = max(a + b for a, b in chunks)

    nc = bacc.Bacc(
        "TRN2",
        target_bir_lowering=False,
        debug=False,
        num_devices=NCORES,
        num_swdge_queues=4,
    )

    xT_in = nc.dram_tensor("xT", [IN_DIM, SHP], F32, kind="ExternalInput")
    w1_in = nc.dram_tensor("w1", [IN_DIM, HID], F32, kind="ExternalInput")
    b1_in = nc.dram_tensor("b1", [1, HID], F32, kind="ExternalInput")
    w2_in = nc.dram_tensor("w2", [HID, NCLS], F32, kind="ExternalInput")
    b2_in = nc.dram_tensor("b2", [1, NCLS], F32, kind="ExternalInput")
    deg_in = nc.dram_tensor("deg", [P, NT], F32, kind="ExternalInput")
    idx_in = nc.dram_tensor("idx", [16, padtot // 16], I16, kind="ExternalInput")
    offs_in = nc.dram_tensor("offs", [P, nch], BF16, kind="ExternalInput")
    iota2_in = nc.dram_tensor("iota2", [1, 2 * P], BF16, kind="ExternalInput")

    h_out = nc.dram_tensor("h_out", [SHP, HID], F32, kind="ExternalOutput")
    ev_out = nc.dram_tensor("ev_out", [SHP, NCLS], F32, kind="ExternalOutput")

    ag1_in = nc.dram_tensor("ag1_in", [SHP, HID], BF16)
    ag1_out = nc.dram_tensor("ag1_out", [NCORES * SHP, HID], BF16)
    ag2_in = nc.dram_tensor("ag2_in", [SHP, NC2], BF16)
    ag2_out = nc.dram_tensor("ag2_out", [NCORES * SHP, NC2], BF16)

    rg = [list(range(NCORES))]
    KQ = [1, 2, 3]  # SWDGE queues for gathers (q0 dispatches serially)
    qctr = [0]

    def gather(out_ap, in_ap, col0, n_chunks, elem):
        q = KQ[qctr[0] % len(KQ)]
        qctr[0] += 1
        nc.gpsimd.dma_gather(
            out_ap=out_ap,
            in_ap=in_ap,
            idxs_ap=idx_sb[:, col0 : col0 + n_chunks * 8],
            num_idxs=n_chunks * P,
            num_idxs_reg=n_chunks * P,
            elem_size=elem,
            single_packet=False,
            queue_num=q,
        )

    with tile.TileContext(nc) as tc:
        with (
            tc.tile_pool(name="const", bufs=1) as cpool,
            tc.tile_pool(name="xw", bufs=3) as xwpool,
            tc.tile_pool(name="work", bufs=3) as wpool,
            tc.tile_pool(name="msgs", bufs=2) as mpool,
            tc.tile_pool(name="oh", bufs=6) as ohpool,
            tc.tile_pool(name="ptr", bufs=2, space="PSUM") as ptr,
            tc.tile_pool(name="pmm", bufs=2, space="PSUM") as pmm,
        ):
            # ---- constants
            ident = cpool.tile([P, P], F32)
            from concourse.masks import make_identity

            make_identity(nc, ident[:])
            iota2_b = cpool.tile([P, 2 * P], BF16)
            nc.gpsimd.dma_start(
                out=iota2_b[:], in_=iota2_in[:].to_broadcast([P, 2 * P])
            )
            b1_b = cpool.tile([P, HID], F32)
            nc.gpsimd.dma_start(out=b1_b[:], in_=b1_in[:].to_broadcast([P, HID]))
            b2_b = cpool.tile([P, NCLS], F32)
            nc.gpsimd.dma_start(out=b2_b[:], in_=b2_in[:].to_broadcast([P, NCLS]))
            w1_sb = cpool.tile([P, IN_DIM // P, HID], BF16)
            for k in range(IN_DIM // P):
                nc.gpsimd.dma_start(
                    out=w1_sb[:, k, :], in_=w1_in[k * P : (k + 1) * P, :]
                )
            w2_sb = cpool.tile([P, HID // P, NCLS], F32)
            for k in range(HID // P):
                nc.sync.dma_start(
                    out=w2_sb[:, k, :], in_=w2_in[k * P : (k + 1) * P, :]
                )
            xT_sb = cpool.tile([P, IN_DIM // P, SHP], BF16)
            for k in range(IN_DIM // P):
                nc.gpsimd.dma_start(
                    out=xT_sb[:, k, :], in_=xT_in[k * P : (k + 1) * P, :]
                )
            deg_sb = cpool.tile([P, NT], F32)
            nc.sync.dma_start(out=deg_sb[:], in_=deg_in[:])
            dinv_r = cpool.tile([P, NT], F32)
            nc.vector.reciprocal(out=dinv_r[:], in_=deg_sb[:])
            dinv = cpool.tile([P, NT], F32)
            nc.scalar.activation(out=dinv[:], in_=dinv_r[:], func=AF.Sqrt)
            idx_sb = cpool.tile([P, padtot // 16], I16)
            for rep in range(8):
                nc.sync.dma_start(
                    out=idx_sb[rep * 16 : (rep + 1) * 16, :], in_=idx_in[:]
                )
            offs_sb = cpool.tile([P, nch], BF16)
            nc.sync.dma_start(out=offs_sb[:], in_=offs_in[:])

            # ---- phase 1: h1s = bf16(dinv * (x @ W1)) -> ag1_in
            for t in range(NT):
                hp = pmm.tile([P, HID], F32, tag="mm")
                for k in range(IN_DIM // P):
                    nc.tensor.matmul(
                        out=hp[:],
                        lhsT=xT_sb[:, k, t * P : (t + 1) * P],
                        rhs=w1_sb[:, k, :],
                        start=(k == 0),
                        stop=(k == IN_DIM // P - 1),
                    )
                h1s = xwpool.tile([P, HID], BF16, tag="h1s")
                nc.scalar.activation(
                    out=h1s[:], in_=hp[:], func=AF.Copy, scale=dinv[:, t : t + 1]
                )
                nc.sync.dma_start(out=ag1_in[t * P : (t + 1) * P, :], in_=h1s[:])

            nc.gpsimd.collective_compute(
                "AllGather",
                mybir.AluOpType.bypass,
                replica_groups=rg,
                ins=[ag1_in.ap().opt()],
                outs=[ag1_out.ap().opt()],
            )

            # ---- phase 2: L1 aggregation + epilogue + L2 transform
            def build_onehots(seg_ch, cht):
                """DVE one-hot chunks (pairs when possible). Returns tiles."""
                tiles = []
                j = 0
                while j < cht:
                    n = 2 if j + 1 < cht else 1
                    oh = ohpool.tile([P, 2, P], BF16, tag="oh")
                    nc.vector.tensor_tensor(
                        out=oh[:, 0:n, :],
                        in0=offs_sb[:, seg_ch + j : seg_ch + j + n]
                        .unsqueeze(2)
                        .to_broadcast([P, n, P]),
                        in1=iota2_b[:].rearrange("p (a b) -> p a b", a=2)[:, 0:n, :],
                        op=mybir.AluOpType.is_equal,
                    )
                    for i in range(n):
                        tiles.append(oh[:, i, :])
                    j += n
                return tiles

            seg_col = 0
            seg_ch = 0
            seg_pos = []
            for t in range(NT):
                ch_lo, ch_hi = chunks[t]
                cht = ch_lo + ch_hi
                seg_pos.append((seg_col, seg_ch))
                msgs = mpool.tile([P, max_cht, HID], BF16, tag="m1")
                if t < 2:
                    nc.gpsimd.memset(msgs[:], 0.0)  # stale-NaN guard (2 slots)
                if ch_lo:
                    gather(msgs[:, 0:ch_lo, :], ag1_out[0:LO_ROWS, :],
                           seg_col, ch_lo, HID)
                if ch_hi:
                    gather(msgs[:, ch_lo:cht, :],
                           ag1_out[LO_ROWS : LO_ROWS + HI_ROWS, :],
                           seg_col + ch_lo * 8, ch_hi, HID)
                agg = pmm.tile([P, HID], F32, tag="mm")
                ohs = build_onehots(seg_ch, cht)
                for j in range(cht):
                    nc.tensor.matmul(
                        out=agg[:],
                        lhsT=ohs[j],
                        rhs=msgs[:, j, :],
                        start=(j == 0),
                        stop=(j == cht - 1),
                    )
                own1 = wpool.tile([P, HID], F32, tag="own1")
                nc.gpsimd.dma_start(
                    out=own1[:], in_=ag1_in[t * P : (t + 1) * P, :]
                )  # bf16 -> f32 cast load of own h1s rows (self loop)
                tsum = wpool.tile([P, HID], F32, tag="tsum")
                nc.vector.tensor_tensor(
                    out=tsum[:], in0=agg[:], in1=own1[:], op=mybir.AluOpType.add
                )
                pre = wpool.tile([P, HID], F32, tag="pre")
                nc.vector.scalar_tensor_tensor(
                    out=pre[:],
                    in0=tsum[:],
                    scalar=dinv[:, t : t + 1],
                    in1=b1_b[:],
                    op0=mybir.AluOpType.mult,
                    op1=mybir.AluOpType.add,
                )
                h_t = wpool.tile([P, HID], F32, tag="ht")
                nc.scalar.activation(out=h_t[:], in_=pre[:], func=AF.Relu)
                nc.sync.dma_start(out=h_out[t * P : (t + 1) * P, :], in_=h_t[:])
                # L2 transform: h2s = bf16(dinv * (h @ W2))
                hT = wpool.tile([P, HID // P, P], F32, tag="hT")
                for k in range(HID // P):
                    tp = ptr.tile([P, P], F32, tag="tr")
                    nc.tensor.transpose(
                        out=tp[:], in_=h_t[:, k * P : (k + 1) * P], identity=ident[:]
                    )
                    nc.scalar.activation(out=hT[:, k, :], in_=tp[:], func=AF.Copy)
                h2p = pmm.tile([P, NCLS], F32, tag="mm2")
                for k in range(HID // P):
                    nc.tensor.matmul(
                        out=h2p[:],
                        lhsT=hT[:, k, :],
                        rhs=w2_sb[:, k, :],
                        start=(k == 0),
                        stop=(k == HID // P - 1),
                    )
                h2s = wpool.tile([P, NCLS], BF16, tag="h2s")
                nc.scalar.activation(
                    out=h2s[:], in_=h2p[:], func=AF.Copy, scale=dinv[:, t : t + 1]
                )
                nc.sync.dma_start(
                    out=ag2_in[t * P : (t + 1) * P, 0:NCLS], in_=h2s[:]
                )
                seg_col += cht * 8
                seg_ch += cht

            nc.gpsimd.collective_compute(
                "AllGather",
                mybir.AluOpType.bypass,
                replica_groups=rg,
                ins=[ag2_in.ap().opt()],
                outs=[ag2_out.ap().opt()],
            )

            # ---- phase 3: L2 aggregation + softplus
            for t in range(NT):
                ch_lo, ch_hi = chunks[t]
                cht = ch_lo + ch_hi
                seg_col, seg_ch = seg_pos[t]
                msgs2 = mpool.tile([P, max_cht, NC2], BF16, tag="m2")
                if t < 2:
                    nc.gpsimd.memset(msgs2[:], 0.0)
                if ch_lo:
                    gather(msgs2[:, 0:ch_lo, :], ag2_out[0:LO_ROWS, :],
                           seg_col, ch_lo, NC2)
                if ch_hi:
                    gather(msgs2[:, ch_lo:cht, :],
                           ag2_out[LO_ROWS : LO_ROWS + HI_ROWS, :],
                           seg_col + ch_lo * 8, ch_hi, NC2)
                agg2 = pmm.tile([P, NCLS], F32, tag="mm2")
                ohs = build_onehots(seg_ch, cht)
                for j in range(cht):
                    nc.tensor.matmul(
                        out=agg2[:],
                        lhsT=ohs[j],
                        rhs=msgs2[:, j, 0:NCLS],
                        start=(j == 0),
                        stop=(j == cht - 1),
                    )
                own2 = wpool.tile([P, NCLS], F32, tag="own2")
                nc.gpsimd.dma_start(
                    out=own2[:], in_=ag2_in[t * P : (t + 1) * P, 0:NCLS]
                )
                t2 = wpool.tile([P, NCLS], F32, tag="t2")
                nc.vector.tensor_tensor(
                    out=t2[:], in0=agg2[:], in1=own2[:], op=mybir.AluOpType.add
                )
                pre2 = wpool.tile([P, NCLS], F32, tag="pre2")
                nc.vector.scalar_tensor_tensor(
                    out=pre2[:],
                    in0=t2[:],
                    scalar=dinv[:, t : t + 1],
                    in1=b2_b[:],
                    op0=mybir.AluOpType.mult,
                    op1=mybir.AluOpType.add,
                )
                # softplus(x) = ln(exp(x) + 1); Exp and Ln share one ACT table
                evx = wpool.tile([P, NCLS], F32, tag="evx")
                nc.scalar.activation(out=evx[:], in_=pre2[:], func=AF.Exp)
                ev = wpool.tile([P, NCLS], F32, tag="ev")
                nc.scalar.activation(out=ev[:], in_=evx[:], func=AF.Ln, bias=1.0)
                nc.sync.dma_start(out=ev_out[t * P : (t + 1) * P, :], in_=ev[:])

    nc.compile()
    return nc


def kernel(x, edge_index, W1, b1, W2, b2, trace=False):
    global LAST_RESULTS
    x = np.asarray(x, dtype=np.float32)
    W1 = np.asarray(W1, dtype=np.float32)
    b1 = np.asarray(b1, dtype=np.float32)
    W2 = np.asarray(W2, dtype=np.float32)
    b2 = np.asarray(b2, dtype=np.float32)

    deg, chunks, idx_w, offs_w = _prep(edge_index)
    nc = _build(chunks)

    iota2 = np.concatenate([np.arange(P), np.arange(P)]).astype(
        ml_dtypes.bfloat16
    ).reshape(1, 2 * P)
    in_maps = []
    for c in range(NCORES):
        x_pad = np.zeros((SHP, IN_DIM), np.float32)
        x_pad[:SH] = x[c * SH : (c + 1) * SH]
        deg_pad = np.ones(SHP, np.float32)
        deg_pad[:SH] = deg[c * SH : (c + 1) * SH]
        deg_w = np.ascontiguousarray(deg_pad.reshape(NT, P).T)
        in_maps.append(
            dict(
                xT=np.ascontiguousarray(x_pad.T),
                w1=W1,
                b1=b1.reshape(1, HID),
                w2=W2,
                b2=b2.reshape(1, NCLS),
                deg=deg_w,
                idx=idx_w[c],
                offs=offs_w[c],
                iota2=iota2,
            )
        )

    if trace:
        import shutil

        shutil.rmtree("/tmp/prof_kernel", ignore_errors=True)
    res = run_bass_kernel_spmd(
        nc,
        in_maps,
        core_ids=list(range(NCORES)),
        trace=trace,
        tmpdir="/tmp/prof_kernel" if trace else None,
    )
    LAST_RESULTS = res

    h = np.concatenate([res.results[c]["h_out"][:SH] for c in range(NCORES)], axis=0)
    ev = np.concatenate(
        [res.results[c]["ev_out"][:SH] for c in range(NCORES)], axis=0
    )
    return ev, h


# revision 10
# speedup vs baseline: 1.9793x; 1.0031x over previous
"""2-layer GCN (EvidentialGNN) on 8 Trainium2 NeuronCores.

Math (per GCNConv with self-loops and symmetric normalization):
    deg[n]  = in-degree(n) + 1          (self loop)
    dinv    = deg ** -0.5
    out[d]  = dinv[d] * ( sum_{e:(s->d)} (dinv[s] * h[s]) + dinv[d]*h[d] ) + b

Key factorization: pre-scale rows h_s[n] = dinv[n] * h[n] BEFORE the
exchange; then out[d] = dinv[d] * (sum_e h_s[src_e] + h_s[d]) + b.
Per-edge work reduces to a pure gather + segmented sum; the self-loop
term reuses the locally kept h_s tile (no gather, no extra DMA).

Distribution (graph/data parallel over 8 cores):
  - nodes sharded by contiguous range (6250/core, padded to 6272).
  - core c computes h1 = x@W1 for its rows (weights replicated, x
    pre-transposed on host so no PE transposes), scales by dinv, casts
    to bf16 and AllGathers the scaled message table; then aggregates
    its incoming edges (dst in shard) by gathering source rows with
    dma_gather (4 SWDGE queues round-robin for parallel descriptor
    generation — descriptor generation is the gather bottleneck at
    ~6.4ns/row/queue-pair) and accumulating per 128-dst tile in fp32
    PSUM via one-hot bf16 matmuls (edges dst-sorted on host, one-hots
    built 8 chunks per DVE op to amortize the ~160ns op overhead).
  - layer 2 repeats at width 64 (table padded to 128 bf16 cols to meet
    the gather's 256B row-granularity); its Exp/Ln softplus epilogue is
    batched (all Exp, then all Ln) to avoid per-tile ACT-table reloads.

Host-side prep is pure integer index manipulation (shard bucketing,
dst-sorting, padding, degree counting) plus layout transposes; all
float math runs on device.
"""

import numpy as np
import ml_dtypes

import concourse.bacc as bacc
import concourse.bass as bass
import concourse.mybir as mybir
import concourse.tile as tile
from concourse.bass_utils import run_bass_kernel_spmd
from concourse.masks import make_identity

P = 128
N_NODES = 50000
NCORES = 8
IN_DIM = 512
HID = 256
NCLS = 64
NC2 = 2 * NCLS  # L2 table padded to 128 cols (gather needs 256B rows)
SH = N_NODES // NCORES  # 6250 rows per core
NT = (SH + P - 1) // P  # 49 dst tiles per core
SHP = NT * P  # 6272 padded rows per core
SPLIT_CORE = 4  # cores 0..3 feed the "lo" gather table
LO_ROWS = SPLIT_CORE * SHP  # 25088 (< 2**15, int16-addressable)
HI_ROWS = NCORES * SHP - LO_ROWS
OHW = 8  # one-hot chunks built per DVE op

F32 = mybir.dt.float32
BF16 = mybir.dt.bfloat16
I16 = mybir.dt.int16
AF = mybir.ActivationFunctionType

# Results of the last kernel() call (for test harness introspection).
LAST_RESULTS = None


def _prep(edge_index):
    """Pure-index host prep (no self-loop edges; handled in epilogue).

    Returns:
      deg:    [N_NODES] float32 (in-degree + 1)
      chunks: list over dst-tile t of (ch_lo, ch_hi) 128-edge chunk counts
              (shared by all cores: max over cores)
      idx_w:  [NCORES, 16, PADTOT//16] int16 gather indices (wrapped);
              padding = 0 (gathers row 0, zeroed by the one-hot)
      offs_w: [NCORES, 128, NCH] bfloat16 local dst offsets (-1 = padding)
    """
    src = np.asarray(edge_index[0]).astype(np.int64)
    dst = np.asarray(edge_index[1]).astype(np.int64)

    deg = (np.bincount(dst, minlength=N_NODES) + 1).astype(np.float32)

    core_of = dst // SH
    per_core = []
    for c in range(NCORES):
        m = core_of == c
        sc = src[m]
        dc = dst[m] - c * SH
        order = np.argsort(dc, kind="stable")
        per_core.append((sc[order], dc[order]))

    cnt = np.zeros((NCORES, NT, 2), np.int64)
    grouped = []
    for c in range(NCORES):
        sc, dc = per_core[c]
        t_of = dc // P
        half_of = (sc >= SPLIT_CORE * SH).astype(np.int64)
        table_row = (sc // SH) * SHP + (sc % SH) - half_of * LO_ROWS
        off = dc - t_of * P
        g = [[None, None] for _ in range(NT)]
        for t in range(NT):
            mt = t_of == t
            for h in (0, 1):
                mh = mt & (half_of == h)
                g[t][h] = (table_row[mh], off[mh])
                cnt[c, t, h] = mh.sum()
        grouped.append(g)

    cmax = cnt.max(axis=0)  # [NT, 2]
    chunks = [
        (int(-(-cmax[t, 0] // P)), int(-(-cmax[t, 1] // P))) for t in range(NT)
    ]
    nch = sum(a + b for a, b in chunks)
    padtot = nch * P

    idx_flat = np.zeros((NCORES, padtot), np.int16)
    offs_flat = np.full((NCORES, padtot), -1.0, np.float32)
    for c in range(NCORES):
        pos = 0
        for t in range(NT):
            for h in (0, 1):
                rows, off = grouped[c][t][h]
                L = chunks[t][h] * P
                n = len(rows)
                # padding keeps idx 0 (valid row, zeroed by one-hot off=-1)
                idx_flat[c, pos : pos + n] = rows.astype(np.int16)
                offs_flat[c, pos : pos + n] = off.astype(np.float32)
                pos += L
        assert pos == padtot

    idx_w = np.ascontiguousarray(
        idx_flat.reshape(NCORES, padtot // 16, 16).transpose(0, 2, 1)
    )
    offs_w = np.ascontiguousarray(
        offs_flat.reshape(NCORES, nch, P).transpose(0, 2, 1)
    ).astype(ml_dtypes.bfloat16)
    return deg, chunks, idx_w, offs_w


def _build(chunks):
    """Build the SPMD Bass program (shared across all 8 cores)."""
    nch = sum(a + b for a, b in chunks)
    padtot = nch * P
    max_cht = max(a + b for a, b in chunks)

    nc = bacc.Bacc(
        "TRN2",
        target_bir_lowering=False,
        debug=False,
        num_devices=NCORES,
        num_swdge_queues=4,
    )

    xT_in = nc.dram_tensor("xT", [IN_DIM, SHP], F32, kind="ExternalInput")
    w1_in = nc.dram_tensor("w1", [IN_DIM, HID], F32, kind="ExternalInput")
    b1_in = nc.dram_tensor("b1", [1, HID], F32, kind="ExternalInput")
    w2_in = nc.dram_tensor("w2", [HID, NCLS], F32, kind="ExternalInput")
    b2_in = nc.dram_tensor("b2", [1, NCLS], F32, kind="ExternalInput")
    deg_in = nc.dram_tensor("deg", [P, NT], F32, kind="ExternalInput")
    idx_in = nc.dram_tensor("idx", [16, padtot // 16], I16, kind="ExternalInput")
    offs_in = nc.dram_tensor("offs", [P, nch], BF16, kind="ExternalInput")
    iota8_in = nc.dram_tensor("iota8", [1, OHW * P], BF16, kind="ExternalInput")

    h_out = nc.dram_tensor("h_out", [SHP, HID], F32, kind="ExternalOutput")
    ev_out = nc.dram_tensor("ev_out", [SHP, NCLS], F32, kind="ExternalOutput")

    ag1_in = nc.dram_tensor("ag1_in", [SHP, HID], BF16)
    ag1_out = nc.dram_tensor("ag1_out", [NCORES * SHP, HID], BF16)
    ag2_in = nc.dram_tensor("ag2_in", [SHP, NC2], BF16)
    ag2_out = nc.dram_tensor("ag2_out", [NCORES * SHP, NC2], BF16)

    rg = [list(range(NCORES))]
    qctr = [0]

    def gather(out_ap, in_ap, col0, n_chunks, elem):
        q = qctr[0] % 4
        qctr[0] += 1
        nc.gpsimd.dma_gather(
            out_ap=out_ap,
            in_ap=in_ap,
            idxs_ap=idx_sb[:, col0 : col0 + n_chunks * 8],
            num_idxs=n_chunks * P,
            num_idxs_reg=n_chunks * P,
            elem_size=elem,
            single_packet=False,
            queue_num=q,
        )

    with tile.TileContext(nc) as tc:
        with (
            tc.tile_pool(name="const", bufs=1) as cpool,
            tc.tile_pool(name="xw", bufs=3) as xwpool,
            tc.tile_pool(name="work", bufs=3) as wpool,
            tc.tile_pool(name="msgs", bufs=2) as mpool,
            tc.tile_pool(name="oh", bufs=4) as ohpool,
            tc.tile_pool(name="ptr", bufs=2, space="PSUM") as ptr,
            tc.tile_pool(name="pmm", bufs=2, space="PSUM") as pmm,
        ):
            # ---- constants
            identb = cpool.tile([P, P], BF16)
            make_identity(nc, identb[:])
            iota8_b = cpool.tile([P, OHW * P], BF16)
            nc.gpsimd.dma_start(
                out=iota8_b[:], in_=iota8_in[:].to_broadcast([P, OHW * P])
            )
            b1_b = cpool.tile([P, HID], F32)
            nc.gpsimd.dma_start(out=b1_b[:], in_=b1_in[:].to_broadcast([P, HID]))
            b2_b = cpool.tile([P, NCLS], F32)
            nc.gpsimd.dma_start(out=b2_b[:], in_=b2_in[:].to_broadcast([P, NCLS]))
            w1_f = cpool.tile([P, IN_DIM // P, HID], F32)
            for k in range(IN_DIM // P):
                nc.sync.dma_start(
                    out=w1_f[:, k, :], in_=w1_in[k * P : (k + 1) * P, :]
                )
            w1_sb = cpool.tile([P, IN_DIM // P, HID], BF16)
            nc.vector.tensor_copy(
                out=w1_sb[:].rearrange("p a b -> p (a b)"),
                in_=w1_f[:].rearrange("p a b -> p (a b)"),
            )
            w2_f = cpool.tile([P, HID // P, NCLS], F32)
            for k in range(HID // P):
                nc.sync.dma_start(
                    out=w2_f[:, k, :], in_=w2_in[k * P : (k + 1) * P, :]
                )
            w2_sb = cpool.tile([P, HID // P, NCLS], BF16)
            nc.vector.tensor_copy(
                out=w2_sb[:].rearrange("p a b -> p (a b)"),
                in_=w2_f[:].rearrange("p a b -> p (a b)"),
            )
            deg_sb = cpool.tile([P, NT], F32)
            nc.sync.dma_start(out=deg_sb[:], in_=deg_in[:])
            dinv_r = cpool.tile([P, NT], F32)
            nc.vector.reciprocal(out=dinv_r[:], in_=deg_sb[:])
            dinv = cpool.tile([P, NT], F32)
            nc.scalar.activation(out=dinv[:], in_=dinv_r[:], func=AF.Sqrt)
            idx_sb = cpool.tile([P, padtot // 16], I16)
            for rep in range(8):
                nc.sync.dma_start(
                    out=idx_sb[rep * 16 : (rep + 1) * 16, :], in_=idx_in[:]
                )
            offs_sb = cpool.tile([P, nch], BF16)
            nc.sync.dma_start(out=offs_sb[:], in_=offs_in[:])
            # persistent per-core row tables (self-loop terms, softplus stage)
            h1s_keep = cpool.tile([P, NT, HID], BF16)
            h2s_keep = cpool.tile([P, NT, NCLS], BF16)
            evx_keep = cpool.tile([P, NT, NCLS], F32)

            xT_r = xT_in.ap().rearrange("(k p) n -> p k n", p=P)

            # ---- phase 1: h1s = bf16(dinv * (x @ W1)) -> ag1_in
            for t in range(NT):
                xt_f = xwpool.tile([P, IN_DIM // P, P], F32, tag="xtf")
                nc.sync.dma_start(
                    out=xt_f[:], in_=xT_r[:, :, t * P : (t + 1) * P]
                )
                xtb = xwpool.tile([P, IN_DIM // P, P], BF16, tag="xtb")
                nc.vector.tensor_copy(
                    out=xtb[:].rearrange("p a b -> p (a b)"),
                    in_=xt_f[:].rearrange("p a b -> p (a b)"),
                )
                hp = pmm.tile([P, HID], F32, tag="mm")
                for k in range(IN_DIM // P):
                    nc.tensor.matmul(
                        out=hp[:],
                        lhsT=xtb[:, k, :],
                        rhs=w1_sb[:, k, :],
                        start=(k == 0),
                        stop=(k == IN_DIM // P - 1),
                    )
                nc.scalar.activation(
                    out=h1s_keep[:, t, :],
                    in_=hp[:],
                    func=AF.Copy,
                    scale=dinv[:, t : t + 1],
                )
                nc.sync.dma_start(
                    out=ag1_in[t * P : (t + 1) * P, :], in_=h1s_keep[:, t, :]
                )

            nc.gpsimd.collective_compute(
                "AllGather",
                mybir.AluOpType.bypass,
                replica_groups=rg,
                ins=[ag1_in.ap().opt()],
                outs=[ag1_out.ap().opt()],
            )

            def build_onehots(seg_ch, cht):
                """One-hot chunks, OHW per DVE op. Returns list of [P,P] APs."""
                tiles = []
                j = 0
                while j < cht:
                    n = min(OHW, cht - j)
                    oh = ohpool.tile([P, OHW, P], BF16, tag="oh")
                    nc.vector.tensor_tensor(
                        out=oh[:, 0:n, :],
                        in0=offs_sb[:, seg_ch + j : seg_ch + j + n]
                        .unsqueeze(2)
                        .to_broadcast([P, n, P]),
                        in1=iota8_b[:].rearrange("p (a b) -> p a b", a=OHW)[
                            :, 0:n, :
                        ],
                        op=mybir.AluOpType.is_equal,
                    )
                    for i in range(n):
                        tiles.append(oh[:, i, :])
                    j += n
                return tiles

            # ---- phase 2: L1 aggregation + epilogue + L2 transform
            seg_col = 0
            seg_ch = 0
            seg_pos = []
            for t in range(NT):
                ch_lo, ch_hi = chunks[t]
                cht = ch_lo + ch_hi
                seg_pos.append((seg_col, seg_ch))
                msgs = mpool.tile([P, max_cht, HID], BF16, tag="m1")
                if t < 2:
                    nc.gpsimd.memset(msgs[:], 0.0)  # stale-NaN guard (2 slots)
                if ch_lo:
                    gather(msgs[:, 0:ch_lo, :], ag1_out[0:LO_ROWS, :],
                           seg_col, ch_lo, HID)
                if ch_hi:
                    gather(msgs[:, ch_lo:cht, :],
                           ag1_out[LO_ROWS : LO_ROWS + HI_ROWS, :],
                           seg_col + ch_lo * 8, ch_hi, HID)
                agg = pmm.tile([P, HID], F32, tag="mm")
                ohs = build_onehots(seg_ch, cht)
                for j in range(cht):
                    nc.tensor.matmul(
                        out=agg[:],
                        lhsT=ohs[j],
                        rhs=msgs[:, j, :],
                        start=(j == 0),
                        stop=(j == cht - 1),
                    )
                tsum = wpool.tile([P, HID], F32, tag="tsum")
                nc.vector.tensor_tensor(
                    out=tsum[:],
                    in0=agg[:],
                    in1=h1s_keep[:, t, :],
                    op=mybir.AluOpType.add,
                )
                pre = wpool.tile([P, HID], F32, tag="pre")
                nc.vector.scalar_tensor_tensor(
                    out=pre[:],
                    in0=tsum[:],
                    scalar=dinv[:, t : t + 1],
                    in1=b1_b[:],
                    op0=mybir.AluOpType.mult,
                    op1=mybir.AluOpType.add,
                )
                h_t = wpool.tile([P, HID], F32, tag="ht")
                nc.scalar.activation(out=h_t[:], in_=pre[:], func=AF.Relu)
                nc.sync.dma_start(out=h_out[t * P : (t + 1) * P, :], in_=h_t[:])
                # L2 transform in bf16: h2s = bf16(dinv * (h @ W2))
                htb = wpool.tile([P, HID], BF16, tag="htb")
                nc.vector.tensor_copy(out=htb[:], in_=h_t[:])
                hTb = wpool.tile([P, HID // P, P], BF16, tag="hTb")
                for k in range(HID // P):
                    tp = ptr.tile([P, P], BF16, tag="tr")
                    nc.tensor.transpose(
                        out=tp[:],
                        in_=htb[:, k * P : (k + 1) * P],
                        identity=identb[:],
                    )
                    nc.vector.tensor_copy(out=hTb[:, k, :], in_=tp[:])
                h2p = pmm.tile([P, NCLS], F32, tag="mm2")
                for k in range(HID // P):
                    nc.tensor.matmul(
                        out=h2p[:],
                        lhsT=hTb[:, k, :],
                        rhs=w2_sb[:, k, :],
                        start=(k == 0),
                        stop=(k == HID // P - 1),
                    )
                nc.scalar.activation(
                    out=h2s_keep[:, t, :],
                    in_=h2p[:],
                    func=AF.Copy,
                    scale=dinv[:, t : t + 1],
                )
                nc.sync.dma_start(
                    out=ag2_in[t * P : (t + 1) * P, 0:NCLS], in_=h2s_keep[:, t, :]
                )
                seg_col += cht * 8
                seg_ch += cht

            nc.gpsimd.collective_compute(
                "AllGather",
                mybir.AluOpType.bypass,
                replica_groups=rg,
                ins=[ag2_in.ap().opt()],
                outs=[ag2_out.ap().opt()],
            )

            # ---- phase 3: L2 aggregation; softplus via batched Exp then Ln
            for t in range(NT):
                ch_lo, ch_hi = chunks[t]
                cht = ch_lo + ch_hi
                seg_col, seg_ch = seg_pos[t]
                msgs2 = mpool.tile([P, max_cht, NC2], BF16, tag="m2")
                if t < 2:
                    nc.gpsimd.memset(msgs2[:], 0.0)
                if ch_lo:
                    gather(msgs2[:, 0:ch_lo, :], ag2_out[0:LO_ROWS, :],
                           seg_col, ch_lo, NC2)
                if ch_hi:
                    gather(msgs2[:, ch_lo:cht, :],
                           ag2_out[LO_ROWS : LO_ROWS + HI_ROWS, :],
                           seg_col + ch_lo * 8, ch_hi, NC2)
                agg2 = pmm.tile([P, NCLS], F32, tag="mm2")
                ohs = build_onehots(seg_ch, cht)
                for j in range(cht):
                    nc.tensor.matmul(
                        out=agg2[:],
                        lhsT=ohs[j],
                        rhs=msgs2[:, j, 0:NCLS],
                        start=(j == 0),
                        stop=(j == cht - 1),
                    )
                t2 = wpool.tile([P, NCLS], F32, tag="t2")
                nc.vector.tensor_tensor(
                    out=t2[:],
                    in0=agg2[:],
                    in1=h2s_keep[:, t, :],
                    op=mybir.AluOpType.add,
                )
                pre2 = wpool.tile([P, NCLS], F32, tag="pre2")
                nc.vector.scalar_tensor_tensor(
                    out=pre2[:],
                    in0=t2[:],
                    scalar=dinv[:, t : t + 1],
                    in1=b2_b[:],
                    op0=mybir.AluOpType.mult,
                    op1=mybir.AluOpType.add,
                )
                nc.scalar.activation(
                    out=evx_keep[:, t, :], in_=pre2[:], func=AF.Exp
                )
            for t in range(NT):
                ev = wpool.tile([P, NCLS], F32, tag="ev")
                nc.scalar.activation(
                    out=ev[:], in_=evx_keep[:, t, :], func=AF.Ln, bias=1.0
                )
                nc.sync.dma_start(out=ev_out[t * P : (t + 1) * P, :], in_=ev[:])

    nc.compile()
    return nc


def kernel(x, edge_index, W1, b1, W2, b2, trace=False):
    global LAST_RESULTS
    x = np.asarray(x, dtype=np.float32)
    W1 = np.asarray(W1, dtype=np.float32)
    b1 = np.asarray(b1, dtype=np.float32)
    W2 = np.asarray(W2, dtype=np.float32)
    b2 = np.asarray(b2, dtype=np.float32)

    deg, chunks, idx_w, offs_w = _prep(edge_index)
    nc = _build(chunks)

    iota8 = np.tile(np.arange(P), OHW).astype(ml_dtypes.bfloat16).reshape(1, OHW * P)
    in_maps = []
    for c in range(NCORES):
        x_pad = np.zeros((SHP, IN_DIM), np.float32)
        x_pad[:SH] = x[c * SH : (c + 1) * SH]
        deg_pad = np.ones(SHP, np.float32)
        deg_pad[:SH] = deg[c * SH : (c + 1) * SH]
        deg_w = np.ascontiguousarray(deg_pad.reshape(NT, P).T)
        in_maps.append(
            dict(
                xT=np.ascontiguousarray(x_pad.T),
                w1=W1,
                b1=b1.reshape(1, HID),
                w2=W2,
                b2=b2.reshape(1, NCLS),
                deg=deg_w,
                idx=idx_w[c],
                offs=offs_w[c],
                iota8=iota8,
            )
        )

    if trace:
        import shutil

        shutil.rmtree("/tmp/prof_kernel", ignore_errors=True)
    res = run_bass_kernel_spmd(
        nc,
        in_maps,
        core_ids=list(range(NCORES)),
        trace=trace,
        tmpdir="/tmp/prof_kernel" if trace else None,
    )
    LAST_RESULTS = res

    h = np.concatenate([res.results[c]["h_out"][:SH] for c in range(NCORES)], axis=0)
    ev = np.concatenate(
        [res.results[c]["ev_out"][:SH] for c in range(NCORES)], axis=0
    )
    return ev, h


# revision 11
# speedup vs baseline: 2.4897x; 1.2579x over previous
"""2-layer GCN (EvidentialGNN) on 8 Trainium2 NeuronCores.

Math (per GCNConv with self-loops and symmetric normalization):
    deg[n]  = in-degree(n) + 1          (self loop)
    dinv    = deg ** -0.5
    out[d]  = dinv[d] * ( sum_{e:(s->d)} (dinv[s] * h[s]) + dinv[d]*h[d] ) + b

Key factorization: pre-scale rows h_s[n] = dinv[n] * h[n] BEFORE the
exchange; then out[d] = dinv[d] * (sum_e h_s[src_e] + h_s[d]) + b.
Per-edge work reduces to a pure gather + segmented sum; the self-loop
term reuses the locally kept h_s tile (no gather, no extra DMA).

Distribution (graph/data parallel over 8 cores):
  - nodes sharded by contiguous range (6250/core, padded to 6272).
  - core c computes h1 = x@W1 for its rows (weights replicated, x
    pre-transposed on host so no PE transposes), scales by dinv, casts
    to bf16 and AllGathers the scaled message table; then aggregates
    its incoming edges (dst in shard) by gathering source rows with
    dma_gather (4 SWDGE queues round-robin for parallel descriptor
    generation — descriptor generation is the gather bottleneck at
    ~6.4ns/row/queue-pair) and accumulating per 128-dst tile in fp32
    PSUM via one-hot bf16 matmuls (edges dst-sorted on host, one-hots
    built 8 chunks per DVE op to amortize the ~160ns op overhead).
  - layer 2 repeats at width 64 (table padded to 128 bf16 cols to meet
    the gather's 256B row-granularity); its Exp/Ln softplus epilogue is
    batched (all Exp, then all Ln) to avoid per-tile ACT-table reloads.

Host-side prep is pure integer index manipulation (shard bucketing,
dst-sorting, padding, degree counting) plus layout transposes; all
float math runs on device.
"""

import numpy as np
import ml_dtypes

import concourse.bacc as bacc
import concourse.bass as bass
import concourse.mybir as mybir
import concourse.tile as tile
from concourse.bass_utils import run_bass_kernel_spmd
from concourse.masks import make_identity

P = 128
N_NODES = 50000
NCORES = 8
IN_DIM = 512
HID = 256
NCLS = 64
NC2 = 2 * NCLS  # L2 table padded to 128 cols (gather needs 256B rows)
SH = N_NODES // NCORES  # 6250 rows per core
NT = (SH + P - 1) // P  # 49 dst tiles per core
SHP = NT * P  # 6272 padded rows per core
SPLIT_CORE = 4  # cores 0..3 feed the "lo" gather table
LO_ROWS = SPLIT_CORE * SHP  # 25088 (< 2**15, int16-addressable)
HI_ROWS = NCORES * SHP - LO_ROWS
OHW = 8  # one-hot chunks built per DVE op

F32 = mybir.dt.float32
BF16 = mybir.dt.bfloat16
I16 = mybir.dt.int16
AF = mybir.ActivationFunctionType

# Results of the last kernel() call (for test harness introspection).
LAST_RESULTS = None


def _prep(edge_index):
    """Pure-index host prep (no self-loop edges; handled in epilogue).

    Returns:
      deg:    [N_NODES] float32 (in-degree + 1)
      chunks: list over dst-tile t of (ch_lo, ch_hi) 128-edge chunk counts
              (shared by all cores: max over cores)
      idx_w:  [NCORES, 16, PADTOT//16] int16 gather indices (wrapped);
              padding = 0 (gathers row 0, zeroed by the one-hot)
      offs_w: [NCORES, 128, NCH] bfloat16 local dst offsets (-1 = padding)
    """
    src = np.asarray(edge_index[0]).astype(np.int64)
    dst = np.asarray(edge_index[1]).astype(np.int64)

    deg = (np.bincount(dst, minlength=N_NODES) + 1).astype(np.float32)

    core_of = dst // SH
    per_core = []
    for c in range(NCORES):
        m = core_of == c
        sc = src[m]
        dc = dst[m] - c * SH
        order = np.argsort(dc, kind="stable")
        per_core.append((sc[order], dc[order]))

    cnt = np.zeros((NCORES, NT, 2), np.int64)
    grouped = []
    for c in range(NCORES):
        sc, dc = per_core[c]
        t_of = dc // P
        half_of = (sc >= SPLIT_CORE * SH).astype(np.int64)
        table_row = (sc // SH) * SHP + (sc % SH) - half_of * LO_ROWS
        off = dc - t_of * P
        g = [[None, None] for _ in range(NT)]
        for t in range(NT):
            mt = t_of == t
            for h in (0, 1):
                mh = mt & (half_of == h)
                g[t][h] = (table_row[mh], off[mh])
                cnt[c, t, h] = mh.sum()
        grouped.append(g)

    cmax = cnt.max(axis=0)  # [NT, 2]
    chunks = [
        (int(-(-cmax[t, 0] // P)), int(-(-cmax[t, 1] // P))) for t in range(NT)
    ]
    nch = sum(a + b for a, b in chunks)
    padtot = nch * P

    idx_flat = np.zeros((NCORES, padtot), np.int16)
    offs_flat = np.full((NCORES, padtot), -1.0, np.float32)
    for c in range(NCORES):
        pos = 0
        for t in range(NT):
            for h in (0, 1):
                rows, off = grouped[c][t][h]
                L = chunks[t][h] * P
                n = len(rows)
                # padding keeps idx 0 (valid row, zeroed by one-hot off=-1)
                idx_flat[c, pos : pos + n] = rows.astype(np.int16)
                offs_flat[c, pos : pos + n] = off.astype(np.float32)
                pos += L
        assert pos == padtot

    idx_w = np.ascontiguousarray(
        idx_flat.reshape(NCORES, padtot // 16, 16).transpose(0, 2, 1)
    )
    offs_w = np.ascontiguousarray(
        offs_flat.reshape(NCORES, nch, P).transpose(0, 2, 1)
    ).astype(ml_dtypes.bfloat16)
    return deg, chunks, idx_w, offs_w


def _build(chunks):
    """Build the SPMD Bass program (shared across all 8 cores)."""
    nch = sum(a + b for a, b in chunks)
    padtot = nch * P
    max_cht = max(a + b for a, b in chunks)

    nc = bacc.Bacc(
        "TRN2",
        target_bir_lowering=False,
        debug=False,
        num_devices=NCORES,
        num_swdge_queues=4,
    )

    xT_in = nc.dram_tensor("xT", [IN_DIM, SHP], F32, kind="ExternalInput")
    w1_in = nc.dram_tensor("w1", [IN_DIM, HID], F32, kind="ExternalInput")
    b1_in = nc.dram_tensor("b1", [1, HID], F32, kind="ExternalInput")
    w2_in = nc.dram_tensor("w2", [HID, NCLS], F32, kind="ExternalInput")
    b2_in = nc.dram_tensor("b2", [1, NCLS], F32, kind="ExternalInput")
    deg_in = nc.dram_tensor("deg", [P, NT], F32, kind="ExternalInput")
    idx_in = nc.dram_tensor("idx", [16, padtot // 16], I16, kind="ExternalInput")
    offs_in = nc.dram_tensor("offs", [P, nch], BF16, kind="ExternalInput")
    iota8_in = nc.dram_tensor("iota8", [1, OHW * P], BF16, kind="ExternalInput")

    h_out = nc.dram_tensor("h_out", [SHP, HID], F32, kind="ExternalOutput")
    ev_out = nc.dram_tensor("ev_out", [SHP, NCLS], F32, kind="ExternalOutput")

    ag1_in = nc.dram_tensor("ag1_in", [SHP, HID], BF16)
    ag1_out = nc.dram_tensor("ag1_out", [NCORES * SHP, HID], BF16, addr_space="Shared")
    ag2_in = nc.dram_tensor("ag2_in", [SHP, NC2], BF16)
    ag2_out = nc.dram_tensor("ag2_out", [NCORES * SHP, NC2], BF16, addr_space="Shared")

    rg = [list(range(NCORES))]
    qctr = [0]

    def gather(out_ap, in_ap, col0, n_chunks, elem):
        q = qctr[0] % 4
        qctr[0] += 1
        nc.gpsimd.dma_gather(
            out_ap=out_ap,
            in_ap=in_ap,
            idxs_ap=idx_sb[:, col0 : col0 + n_chunks * 8],
            num_idxs=n_chunks * P,
            num_idxs_reg=n_chunks * P,
            elem_size=elem,
            single_packet=False,
            queue_num=q,
        )

    with tile.TileContext(nc) as tc:
        with (
            tc.tile_pool(name="const", bufs=1) as cpool,
            tc.tile_pool(name="xw", bufs=3) as xwpool,
            tc.tile_pool(name="work", bufs=3) as wpool,
            tc.tile_pool(name="msgs", bufs=4) as mpool,
            tc.tile_pool(name="oh", bufs=4) as ohpool,
            tc.tile_pool(name="ptr", bufs=2, space="PSUM") as ptr,
            tc.tile_pool(name="pmm", bufs=2, space="PSUM") as pmm,
        ):
            # ---- constants
            identb = cpool.tile([P, P], BF16)
            make_identity(nc, identb[:])
            iota8_b = cpool.tile([P, OHW * P], BF16)
            nc.gpsimd.dma_start(
                out=iota8_b[:], in_=iota8_in[:].to_broadcast([P, OHW * P])
            )
            b1_b = cpool.tile([P, HID], F32)
            nc.gpsimd.dma_start(out=b1_b[:], in_=b1_in[:].to_broadcast([P, HID]))
            b2_b = cpool.tile([P, NCLS], F32)
            nc.gpsimd.dma_start(out=b2_b[:], in_=b2_in[:].to_broadcast([P, NCLS]))
            w1_f = cpool.tile([P, IN_DIM // P, HID], F32)
            for k in range(IN_DIM // P):
                nc.sync.dma_start(
                    out=w1_f[:, k, :], in_=w1_in[k * P : (k + 1) * P, :]
                )
            w2_f = cpool.tile([P, HID // P, NCLS], F32)
            for k in range(HID // P):
                nc.sync.dma_start(
                    out=w2_f[:, k, :], in_=w2_in[k * P : (k + 1) * P, :]
                )
            w2_sb = cpool.tile([P, HID // P, NCLS], BF16)
            nc.vector.tensor_copy(
                out=w2_sb[:].rearrange("p a b -> p (a b)"),
                in_=w2_f[:].rearrange("p a b -> p (a b)"),
            )
            deg_sb = cpool.tile([P, NT], F32)
            nc.sync.dma_start(out=deg_sb[:], in_=deg_in[:])
            dinv_r = cpool.tile([P, NT], F32)
            nc.vector.reciprocal(out=dinv_r[:], in_=deg_sb[:])
            dinv = cpool.tile([P, NT], F32)
            nc.scalar.activation(out=dinv[:], in_=dinv_r[:], func=AF.Sqrt)
            idx_sb = cpool.tile([P, padtot // 16], I16)
            for rep in range(8):
                nc.sync.dma_start(
                    out=idx_sb[rep * 16 : (rep + 1) * 16, :], in_=idx_in[:]
                )
            offs_sb = cpool.tile([P, nch], BF16)
            nc.sync.dma_start(out=offs_sb[:], in_=offs_in[:])
            # persistent per-core row tables (self-loop terms, softplus stage)
            h1s_keep = cpool.tile([P, NT, HID], BF16)
            h2s_keep = cpool.tile([P, NT, NCLS], BF16)
            evx_keep = cpool.tile([P, NT, NCLS], F32)

            xT_r = xT_in.ap().rearrange("(k p) n -> p k n", p=P)

            # ---- phase 1: h1s = bf16(dinv * (x @ W1)) -> ag1_in
            for t in range(NT):
                xt_f = xwpool.tile([P, IN_DIM // P, P], F32, tag="xtf")
                nc.sync.dma_start(
                    out=xt_f[:], in_=xT_r[:, :, t * P : (t + 1) * P]
                )
                hp = pmm.tile([P, HID], F32, tag="mm")
                for k in range(IN_DIM // P):
                    nc.tensor.matmul(
                        out=hp[:],
                        lhsT=xt_f[:, k, :],
                        rhs=w1_f[:, k, :],
                        start=(k == 0),
                        stop=(k == IN_DIM // P - 1),
                    )
                nc.scalar.activation(
                    out=h1s_keep[:, t, :],
                    in_=hp[:],
                    func=AF.Copy,
                    scale=dinv[:, t : t + 1],
                )
                nc.sync.dma_start(
                    out=ag1_in[t * P : (t + 1) * P, :], in_=h1s_keep[:, t, :]
                )

            nc.gpsimd.collective_compute(
                "AllGather",
                mybir.AluOpType.bypass,
                replica_groups=rg,
                ins=[ag1_in.ap().opt()],
                outs=[ag1_out.ap().opt()],
            )

            def build_onehots(seg_ch, cht):
                """One-hot chunks, OHW per DVE op. Returns list of [P,P] APs."""
                tiles = []
                j = 0
                while j < cht:
                    n = min(OHW, cht - j)
                    oh = ohpool.tile([P, OHW, P], BF16, tag="oh")
                    nc.vector.tensor_tensor(
                        out=oh[:, 0:n, :],
                        in0=offs_sb[:, seg_ch + j : seg_ch + j + n]
                        .unsqueeze(2)
                        .to_broadcast([P, n, P]),
                        in1=iota8_b[:].rearrange("p (a b) -> p a b", a=OHW)[
                            :, 0:n, :
                        ],
                        op=mybir.AluOpType.is_equal,
                    )
                    for i in range(n):
                        tiles.append(oh[:, i, :])
                    j += n
                return tiles

            # ---- phase 2: L1 aggregation + epilogue + L2 transform
            seg_col = 0
            seg_ch = 0
            seg_pos = []
            for t in range(NT):
                ch_lo, ch_hi = chunks[t]
                cht = ch_lo + ch_hi
                seg_pos.append((seg_col, seg_ch))
                msgs = mpool.tile([P, max_cht, HID], BF16, tag="m1")
                if t < 4:
                    nc.gpsimd.memset(msgs[:], 0.0)  # stale-NaN guard (2 slots)
                if ch_lo:
                    gather(msgs[:, 0:ch_lo, :], ag1_out[0:LO_ROWS, :],
                           seg_col, ch_lo, HID)
                if ch_hi:
                    gather(msgs[:, ch_lo:cht, :],
                           ag1_out[LO_ROWS : LO_ROWS + HI_ROWS, :],
                           seg_col + ch_lo * 8, ch_hi, HID)
                agg = pmm.tile([P, HID], F32, tag="mm")
                ohs = build_onehots(seg_ch, cht)
                for j in range(cht):
                    nc.tensor.matmul(
                        out=agg[:],
                        lhsT=ohs[j],
                        rhs=msgs[:, j, :],
                        start=(j == 0),
                        stop=(j == cht - 1),
                    )
                tsum = wpool.tile([P, HID], F32, tag="tsum")
                nc.vector.tensor_tensor(
                    out=tsum[:],
                    in0=agg[:],
                    in1=h1s_keep[:, t, :],
                    op=mybir.AluOpType.add,
                )
                pre = wpool.tile([P, HID], F32, tag="pre")
                nc.vector.scalar_tensor_tensor(
                    out=pre[:],
                    in0=tsum[:],
                    scalar=dinv[:, t : t + 1],
                    in1=b1_b[:],
                    op0=mybir.AluOpType.mult,
                    op1=mybir.AluOpType.add,
                )
                h_t = wpool.tile([P, HID], F32, tag="ht")
                nc.scalar.activation(out=h_t[:], in_=pre[:], func=AF.Relu)
                nc.sync.dma_start(out=h_out[t * P : (t + 1) * P, :], in_=h_t[:])
                # L2 transform in bf16: h2s = bf16(dinv * (h @ W2))
                htb = wpool.tile([P, HID], BF16, tag="htb")
                nc.scalar.activation(out=htb[:], in_=pre[:], func=AF.Relu)
                hTb = wpool.tile([P, HID // P, P], BF16, tag="hTb")
                for k in range(HID // P):
                    tp = ptr.tile([P, P], BF16, tag="tr")
                    nc.tensor.transpose(
                        out=tp[:],
                        in_=htb[:, k * P : (k + 1) * P],
                        identity=identb[:],
                    )
                    nc.vector.tensor_copy(out=hTb[:, k, :], in_=tp[:])
                h2p = pmm.tile([P, NCLS], F32, tag="mm2")
                for k in range(HID // P):
                    nc.tensor.matmul(
                        out=h2p[:],
                        lhsT=hTb[:, k, :],
                        rhs=w2_sb[:, k, :],
                        start=(k == 0),
                        stop=(k == HID // P - 1),
                    )
                nc.scalar.activation(
                    out=h2s_keep[:, t, :],
                    in_=h2p[:],
                    func=AF.Copy,
                    scale=dinv[:, t : t + 1],
                )
                nc.sync.dma_start(
                    out=ag2_in[t * P : (t + 1) * P, 0:NCLS], in_=h2s_keep[:, t, :]
                )
                seg_col += cht * 8
                seg_ch += cht

            nc.gpsimd.collective_compute(
                "AllGather",
                mybir.AluOpType.bypass,
                replica_groups=rg,
                ins=[ag2_in.ap().opt()],
                outs=[ag2_out.ap().opt()],
            )

            # ---- phase 3: L2 aggregation; softplus via batched Exp then Ln
            for t in range(NT):
                ch_lo, ch_hi = chunks[t]
                cht = ch_lo + ch_hi
                seg_col, seg_ch = seg_pos[t]
                msgs2 = mpool.tile([P, max_cht, NC2], BF16, tag="m2")
                if t < 4:
                    nc.gpsimd.memset(msgs2[:], 0.0)
                if ch_lo:
                    gather(msgs2[:, 0:ch_lo, :], ag2_out[0:LO_ROWS, :],
                           seg_col, ch_lo, NC2)
                if ch_hi:
                    gather(msgs2[:, ch_lo:cht, :],
                           ag2_out[LO_ROWS : LO_ROWS + HI_ROWS, :],
                           seg_col + ch_lo * 8, ch_hi, NC2)
                agg2 = pmm.tile([P, NCLS], F32, tag="mm2")
                ohs = build_onehots(seg_ch, cht)
                for j in range(cht):
                    nc.tensor.matmul(
                        out=agg2[:],
                        lhsT=ohs[j],
                        rhs=msgs2[:, j, 0:NCLS],
                        start=(j == 0),
                        stop=(j == cht - 1),
                    )
                t2 = wpool.tile([P, NCLS], F32, tag="t2")
                nc.vector.tensor_tensor(
                    out=t2[:],
                    in0=agg2[:],
                    in1=h2s_keep[:, t, :],
                    op=mybir.AluOpType.add,
                )
                pre2 = wpool.tile([P, NCLS], F32, tag="pre2")
                nc.vector.scalar_tensor_tensor(
                    out=pre2[:],
                    in0=t2[:],
                    scalar=dinv[:, t : t + 1],
                    in1=b2_b[:],
                    op0=mybir.AluOpType.mult,
                    op1=mybir.AluOpType.add,
                )
                nc.scalar.activation(
                    out=evx_keep[:, t, :], in_=pre2[:], func=AF.Exp
                )
            ev_all = cpool.tile([P, NT, NCLS], F32)
            nc.scalar.activation(
                out=ev_all[:].rearrange("p a b -> p (a b)"),
                in_=evx_keep[:].rearrange("p a b -> p (a b)"),
                func=AF.Ln,
                bias=1.0,
            )
            nc.sync.dma_start(
                out=ev_out.ap().rearrange("(t p) c -> p t c", p=P), in_=ev_all[:]
            )

    nc.compile()
    return nc


def kernel(x, edge_index, W1, b1, W2, b2, trace=False):
    global LAST_RESULTS
    x = np.asarray(x, dtype=np.float32)
    W1 = np.asarray(W1, dtype=np.float32)
    b1 = np.asarray(b1, dtype=np.float32)
    W2 = np.asarray(W2, dtype=np.float32)
    b2 = np.asarray(b2, dtype=np.float32)

    deg, chunks, idx_w, offs_w = _prep(edge_index)
    nc = _build(chunks)

    iota8 = np.tile(np.arange(P), OHW).astype(ml_dtypes.bfloat16).reshape(1, OHW * P)
    in_maps = []
    for c in range(NCORES):
        x_pad = np.zeros((SHP, IN_DIM), np.float32)
        x_pad[:SH] = x[c * SH : (c + 1) * SH]
        deg_pad = np.ones(SHP, np.float32)
        deg_pad[:SH] = deg[c * SH : (c + 1) * SH]
        deg_w = np.ascontiguousarray(deg_pad.reshape(NT, P).T)
        in_maps.append(
            dict(
                xT=np.ascontiguousarray(x_pad.T),
                w1=W1,
                b1=b1.reshape(1, HID),
                w2=W2,
                b2=b2.reshape(1, NCLS),
                deg=deg_w,
                idx=idx_w[c],
                offs=offs_w[c],
                iota8=iota8,
            )
        )

    if trace:
        import shutil

        shutil.rmtree("/tmp/prof_kernel", ignore_errors=True)
    res = run_bass_kernel_spmd(
        nc,
        in_maps,
        core_ids=list(range(NCORES)),
        trace=trace,
        tmpdir="/tmp/prof_kernel" if trace else None,
    )
    LAST_RESULTS = res

    h = np.concatenate([res.results[c]["h_out"][:SH] for c in range(NCORES)], axis=0)
    ev = np.concatenate(
        [res.results[c]["ev_out"][:SH] for c in range(NCORES)], axis=0
    )
    return ev, h
